# revision 6
# baseline (speedup 1.0000x reference)
"""Trainium2 Bass kernel for nn_GAT_39427799777563 (GAT message passing).

Math (per item row n, K=32 neighbors, D=100 dims):
    We   = entity_embs * w_r                  # [K, D] elementwise
    e_k  = sum_d We[k, d]                     # neighbor logits
    p_k  = adj_k * exp(leaky_relu(e_k))       # masked unnormalized softmax
    h'   = sum_k p_k * We[k, :]               # weighted neighbor sum (unnorm.)
    x    = (h' @ W_out.T) / sum_k p_k + (b_out + item_embs)

The max-subtraction in the reference softmax is dropped: e is bounded
(|e| < ~70 for this distribution) so exp(e) and the unnormalized h'
stay comfortably inside fp32 range, and softmax is shift-invariant.
Masking happens after exp (exp of the -9e15 fill is exactly 0), fused
into a tensor_tensor_reduce that also produces the denominator. The
1/denominator scale and the bias+residual add are folded into one
scalar_tensor_tensor on the matmul output (item_embs + b_out is
precomputed on host).

Sharding: pure data parallel over N across 8 cores; rows padded
40000 -> 40960 so every core runs 40 full 128-row tiles.

Engine split per 128-row tile (fp32 everywhere; all ~balanced):
    DVE : We mul, 8/32 of the e-sums (one strided reduce), leaky-relu,
          mask+denominator (TTR), reciprocal, p-broadcast weighting mul
          (in place), strided k-reduction, fused scale+residual epilogue
    ACT : 24/32 e-sums via activation(Copy, accum_out), exp,
          PSUM->SBUF copy of transposed h'
    PE  : h' transpose + the 100x100 linear
    SP  : all DMA (HWDGE)
"""

from contextlib import ExitStack

import numpy as np

import concourse.bass as bass
import concourse.bacc as bacc
import concourse.mybir as mybir
import concourse.tile as tile

F32 = mybir.dt.float32
ALPHA = 0.2

N, K, D = 40000, 32, 100
N_CORES = 8
P = 128            # rows per tile == SBUF partitions
import os as _os
M_DVE = int(_os.environ.get("GAT_M_DVE", "8"))  # k's d-summed on DVE; rest on ACT
STORE_CHUNK = 8    # tiles per output store


def build(n_tiles: int, repeats: int = 1, mode: str = "full"):
    """Build the per-core Bass program for n_tiles 128-row tiles.

    repeats > 1 wraps the whole tile loop in a hardware For_i loop that
    re-executes it, for dispatch-overhead-free benchmarking.
    mode: "full" | "dma" | "dve" | "act"  (ablation variants for perf
    attribution; only "full" computes the real output)."""
    rows = n_tiles * P
    nc = bacc.Bacc("TRN2", target_bir_lowering=False, debug=False,
                   num_devices=N_CORES)

    ent_d = nc.dram_tensor("ent", [rows, K * D], F32, kind="ExternalInput")
    wr_d = nc.dram_tensor("wr", [rows, K * D], F32, kind="ExternalInput")
    # pre-swizzled on host to [128, n_tiles * K] / [128, n_tiles * D]
    adj_d = nc.dram_tensor("adjf", [P, n_tiles * K], F32, kind="ExternalInput")
    itemb_d = nc.dram_tensor("itemb", [P, n_tiles * D], F32, kind="ExternalInput")
    wt_d = nc.dram_tensor("wt", [D, D], F32, kind="ExternalInput")   # W_out.T
    ident_d = nc.dram_tensor("ident", [P, P], F32, kind="ExternalInput")
    out_d = nc.dram_tensor("out", [P, n_tiles * D], F32, kind="ExternalOutput")

    AF = mybir.ActivationFunctionType
    AL = mybir.AluOpType
    AX = mybir.AxisListType

    with tile.TileContext(nc) as tc, ExitStack() as ctx:
        const = ctx.enter_context(tc.tile_pool(name="const", bufs=1))
        nbig = 2 if (mode == "full" and n_tiles % 2 == 0) else 3
        big = ctx.enter_context(tc.tile_pool(name="big", bufs=nbig))
        wep = ctx.enter_context(tc.tile_pool(name="wep", bufs=2))
        small = ctx.enter_context(tc.tile_pool(name="small", bufs=2))
        psum = ctx.enter_context(tc.tile_pool(name="psum", bufs=2, space="PSUM"))

        # resident constants / accumulators
        adjf = const.tile([P, n_tiles * K], F32)
        itemb = const.tile([P, n_tiles * D], F32)
        wt = const.tile([D, D], F32)
        ident = const.tile([P, P], F32)
        out_all = const.tile([P, n_tiles * D], F32)
        nc.sync.dma_start(adjf[:], adj_d[:])
        nc.sync.dma_start(itemb[:], itemb_d[:])
        nc.sync.dma_start(wt[:], wt_d[:])
        nc.sync.dma_start(ident[:], ident_d[:])

        def tile_loop():
            fn = body_pairs if (mode == "full" and n_tiles % 2 == 0) else body
            fn(nc, tc, n_tiles, ent_d, wr_d, out_d,
               adjf, itemb, wt, ident, out_all, big, wep, small, psum,
               mode=mode)

        if repeats > 1:
            with tc.For_i(0, repeats, 1):
                tile_loop()
        else:
            tile_loop()

    nc.compile()
    return nc


def body(nc, tc, n_tiles, ent_d, wr_d, out_d, adjf, itemb, wt, ident,
         out_all, big, wep, small, psum, mode="full"):
    AF = mybir.ActivationFunctionType
    AL = mybir.AluOpType
    AX = mybir.AxisListType
    if True:
        for t in range(n_tiles):
            rsl = slice(t * P, (t + 1) * P)

            ent_t = big.tile([P, K * D], F32, tag="ent")
            nc.sync.dma_start(ent_t[:], ent_d[rsl, :])
            wr_t = big.tile([P, K * D], F32, tag="wr")
            nc.sync.dma_start(wr_t[:], wr_d[rsl, :])

            if mode == "dma":
                nc.vector.tensor_copy(out_all[:, t * D:(t + 1) * D],
                                      ent_t[:, :D])
                if (t + 1) % STORE_CHUNK == 0:
                    csl = slice((t + 1 - STORE_CHUNK) * D, (t + 1) * D)
                    nc.sync.dma_start(out_d[:, csl], out_all[:, csl])
                continue

            # We = ent * wr   (DVE, one big 1x pass)
            we = wep.tile([P, K * D], F32, tag="we")
            nc.vector.tensor_mul(we[:], ent_t[:], wr_t[:])

            # e_k = sum_d We[k, :]: k < m_dve on DVE (one strided reduce),
            # the rest on ACT (accumulate, in-place copy)
            m_dve = K if mode == "dve" else M_DVE
            e = small.tile([P, K], F32, tag="e")
            nc.vector.tensor_reduce(
                e[:, :m_dve],
                we[:, :m_dve * D].rearrange("p (k d) -> p k d", k=m_dve),
                axis=AX.X, op=AL.add,
            )
            for k in range(m_dve, K):
                ksl = slice(k * D, (k + 1) * D)
                nc.scalar.activation(we[:, ksl], we[:, ksl], AF.Copy,
                                     accum_out=e[:, k:k + 1])

            if mode == "act":
                ex = small.tile([P, K], F32, tag="ex")
                nc.scalar.activation(ex[:], e[:], AF.Exp)
                nc.vector.tensor_copy(out_all[:, t * D:(t + 1) * D],
                                      ex[:].unsqueeze(-1)
                                      .broadcast_to([P, K, D])[:, 0, :D])
                if (t + 1) % STORE_CHUNK == 0:
                    csl = slice((t + 1 - STORE_CHUNK) * D, (t + 1) * D)
                    nc.sync.dma_start(out_d[:, csl], out_all[:, csl])
                continue

            # leaky relu (DVE): elr = max(alpha*e, e)
            elr = small.tile([P, K], F32, tag="elr")
            nc.vector.scalar_tensor_tensor(elr[:], e[:], ALPHA, e[:],
                                           op0=AL.mult, op1=AL.max)

            # exp (ACT); in dve-ablation keep it on DVE instead
            ex = small.tile([P, K], F32, tag="ex")
            if mode == "dve":
                nc.vector.tensor_copy(ex[:], elr[:])
            else:
                nc.scalar.activation(ex[:], elr[:], AF.Exp)

            # p = ex * adj ; sumexp = sum_k p
            # (tensor_tensor_reduce would fuse these, but InstTensorTensorReduce
            #  is broken on this execution path — NRT_EXEC_UNIT_UNRECOVERABLE)
            p = small.tile([P, K], F32, tag="p")
            sumexp = small.tile([P, 1], F32, tag="sumexp")
            nc.vector.tensor_mul(p[:], ex[:], adjf[:, t * K:(t + 1) * K])
            nc.vector.reduce_sum(sumexp[:], p[:], axis=AX.X)
            rs = small.tile([P, 1], F32, tag="rs")
            nc.vector.reciprocal(rs[:], sumexp[:])

            # We *= p (k-broadcast over d), in place (DVE)
            we3 = we[:].rearrange("p (k d) -> p k d", k=K)
            p3 = p[:].unsqueeze(-1).broadcast_to([P, K, D])
            nc.vector.tensor_mul(we3, we3, p3)

            # h'_u[d] = sum_k We[k, d]  (DVE strided reduce, innermost=k)
            hu = small.tile([P, D], F32, tag="hu")
            nc.vector.tensor_reduce(
                hu[:], we[:].rearrange("p (k d) -> p d k", k=K),
                axis=AX.X, op=AL.add,
            )

            if mode == "dve":
                nc.vector.scalar_tensor_tensor(
                    out_all[:, t * D:(t + 1) * D], hu[:], rs[:],
                    itemb[:, t * D:(t + 1) * D], op0=AL.mult, op1=AL.add,
                )
            else:
                # transpose h'_u -> [D, P] (PE), copy PSUM->SBUF (ACT)
                ht_ps = psum.tile([D, P], F32, tag="htp")
                nc.tensor.transpose(ht_ps[:], hu[:], ident[:])
                ht = small.tile([D, P], F32, tag="ht")
                nc.scalar.copy(ht[:], ht_ps[:])

                # x_mm = h'_u @ W_out.T  (PE)
                x_ps = psum.tile([P, D], F32, tag="xps")
                nc.tensor.matmul(x_ps[:], ht[:], wt[:], start=True, stop=True)

                # out = x_mm * (1/sumexp) + (item + b)  (DVE fused epilogue)
                nc.vector.scalar_tensor_tensor(
                    out_all[:, t * D:(t + 1) * D], x_ps[:], rs[:],
                    itemb[:, t * D:(t + 1) * D], op0=AL.mult, op1=AL.add,
                )

            if (t + 1) % STORE_CHUNK == 0:
                csl = slice((t + 1 - STORE_CHUNK) * D, (t + 1) * D)
                nc.sync.dma_start(out_d[:, csl], out_all[:, csl])

        rem = n_tiles % STORE_CHUNK
        if rem:
            csl = slice((n_tiles - rem) * D, n_tiles * D)
            nc.sync.dma_start(out_d[:, csl], out_all[:, csl])


def _shard_host(item_embs, entity_embs, w_r, adj, W_out, b_out, n_tiles):
    """Pad + shard + swizzle the full inputs into 8 per-core input maps."""
    rows = n_tiles * P
    n_pad = N_CORES * rows

    ent = np.ascontiguousarray(np.asarray(entity_embs, np.float32).reshape(N, K * D))
    wr = np.ascontiguousarray(np.asarray(w_r, np.float32).reshape(N, K * D))
    adjf = np.asarray(adj).astype(np.float32)
    itemb = np.asarray(item_embs, np.float32) + np.asarray(b_out, np.float32)

    pad = n_pad - N
    ent = np.pad(ent, ((0, pad), (0, 0)))
    wr = np.pad(wr, ((0, pad), (0, 0)))
    # padding rows get adj=1 so the softmax denominator stays nonzero
    adjf = np.pad(adjf, ((0, pad), (0, 0)), constant_values=1.0)
    itemb = np.pad(itemb, ((0, pad), (0, 0)))

    wt = np.ascontiguousarray(np.asarray(W_out, np.float32).T)
    ident = np.eye(P, dtype=np.float32)

    in_maps = []
    for c in range(N_CORES):
        rs = slice(c * rows, (c + 1) * rows)
        a_sw = np.ascontiguousarray(
            adjf[rs].reshape(n_tiles, P, K).transpose(1, 0, 2).reshape(P, n_tiles * K))
        i_sw = np.ascontiguousarray(
            itemb[rs].reshape(n_tiles, P, D).transpose(1, 0, 2).reshape(P, n_tiles * D))
        in_maps.append({
            "ent": np.ascontiguousarray(ent[rs]),
            "wr": np.ascontiguousarray(wr[rs]),
            "adjf": a_sw,
            "itemb": i_sw,
            "wt": wt,
            "ident": ident,
        })
    return in_maps


def _unshard_host(results, n_tiles):
    rows = n_tiles * P
    outs = []
    for c in range(N_CORES):
        o = results[c]["out"]  # [P, n_tiles * D] swizzled
        outs.append(o.reshape(P, n_tiles, D).transpose(1, 0, 2).reshape(rows, D))
    return np.concatenate(outs)[:N]


_N_TILES_FULL = 40  # 8 cores * 40 tiles * 128 rows = 40960 >= 40000


def kernel(item_embs, entity_embs, w_r, adj, W_out, b_out):
    from concourse.bass_utils import run_bass_kernel_spmd

    nc = build(_N_TILES_FULL)
    in_maps = _shard_host(item_embs, entity_embs, w_r, adj, W_out, b_out,
                          _N_TILES_FULL)
    res = run_bass_kernel_spmd(nc, in_maps, core_ids=list(range(N_CORES)))
    return _unshard_host(res.results, _N_TILES_FULL).astype(np.float32)


def body_pairs(nc, tc, n_tiles, ent_d, wr_d, out_d, adjf, itemb, wt, ident,
               out_all, big, wep, small, psum, mode="full"):
    """Two 128-row tiles per step: halves instruction count on the big DVE
    passes and the softmax smalls (per-op overhead + DRAIN dominate on HW)."""
    AF = mybir.ActivationFunctionType
    AL = mybir.AluOpType
    AX = mybir.AxisListType
    J = 2
    F = K * D
    for pg in range(n_tiles // 2):
        rsl = slice(pg * J * P, (pg + 1) * J * P)
        ent_t = big.tile([P, J * F], F32, tag="ent")
        nc.sync.dma_start(ent_t[:].rearrange("p (j f) -> p j f", j=J),
                          ent_d[rsl, :].rearrange("(j p) f -> p j f", p=P))
        wr_t = big.tile([P, J * F], F32, tag="wr")
        nc.sync.dma_start(wr_t[:].rearrange("p (j f) -> p j f", j=J),
                          wr_d[rsl, :].rearrange("(j p) f -> p j f", p=P))

        we = wep.tile([P, J * F], F32, tag="we")
        nc.vector.tensor_mul(we[:], ent_t[:], wr_t[:])

        e = small.tile([P, J * K], F32, tag="e")
        nc.vector.tensor_reduce(
            e[:].rearrange("p (j k) -> p j k", j=J)[:, :, :M_DVE],
            we[:].rearrange("p (j k d) -> p j k d", j=J, k=K)[:, :, :M_DVE, :],
            axis=AX.X, op=AL.add,
        )
        for j in range(J):
            for k in range(M_DVE, K):
                ksl = slice(j * F + k * D, j * F + (k + 1) * D)
                nc.scalar.activation(we[:, ksl], we[:, ksl], AF.Copy,
                                     accum_out=e[:, j * K + k:j * K + k + 1])

        elr = small.tile([P, J * K], F32, tag="elr")
        nc.vector.scalar_tensor_tensor(elr[:], e[:], ALPHA, e[:],
                                       op0=AL.mult, op1=AL.max)
        ex = small.tile([P, J * K], F32, tag="ex")
        nc.scalar.activation(ex[:], elr[:], AF.Exp)

        p = small.tile([P, J * K], F32, tag="p")
        sumexp = small.tile([P, J], F32, tag="sumexp")
        nc.vector.tensor_mul(p[:], ex[:], adjf[:, pg * J * K:(pg + 1) * J * K])
        nc.vector.tensor_reduce(sumexp[:],
                                p[:].rearrange("p (j k) -> p j k", j=J),
                                axis=AX.X, op=AL.add)
        rs = small.tile([P, J], F32, tag="rs")
        nc.vector.reciprocal(rs[:], sumexp[:])

        we4 = we[:].rearrange("p (j k d) -> p j k d", j=J, k=K)
        p4 = (p[:].rearrange("p (j k) -> p j k", j=J)
              .unsqueeze(-1).broadcast_to([P, J, K, D]))
        nc.vector.tensor_mul(we4, we4, p4)

        hu = small.tile([P, J * D], F32, tag="hu")
        nc.vector.tensor_reduce(
            hu[:].rearrange("p (j d) -> p j d", j=J),
            we[:].rearrange("p (j k d) -> p j d k", j=J, k=K),
            axis=AX.X, op=AL.add,
        )

        for j in range(J):
            t = pg * J + j
            ht_ps = psum.tile([D, P], F32, tag="htp")
            nc.tensor.transpose(ht_ps[:], hu[:, j * D:(j + 1) * D], ident[:])
            ht = small.tile([D, P], F32, tag="ht")
            nc.scalar.copy(ht[:], ht_ps[:])
            x_ps = psum.tile([P, D], F32, tag="xps")
            nc.tensor.matmul(x_ps[:], ht[:], wt[:], start=True, stop=True)
            nc.vector.scalar_tensor_tensor(
                out_all[:, t * D:(t + 1) * D], x_ps[:], rs[:, j:j + 1],
                itemb[:, t * D:(t + 1) * D], op0=AL.mult, op1=AL.add,
            )

        if (pg + 1) % (STORE_CHUNK // 2) == 0:
            csl = slice((pg + 1 - STORE_CHUNK // 2) * J * D,
                        (pg + 1) * J * D)
            nc.sync.dma_start(out_d[:, csl], out_all[:, csl])

    n_pairs = n_tiles // 2
    rem = n_pairs % (STORE_CHUNK // 2)
    if rem:
        csl = slice((n_pairs - rem) * J * D, n_pairs * J * D)
        nc.sync.dma_start(out_d[:, csl], out_all[:, csl])

